# revision 23
# baseline (speedup 1.0000x reference)
"""Trainium2 Bass kernel for the CausalComp GNN message-passing rollout.

Data-parallel over batch B=128 across 8 NeuronCores (16 batches/core).
Per-core row dim = 16 batches x 8 slots = 128 = SBUF partition count.

Key algebraic restructurings vs the reference einsums:
  * pair = concat(slots_i, slots_j) @ W  ==  slots@W_top (A) + slots@W_bot (B,
    sender-broadcast).  A/B are computed once per message-passing iteration as
    [128, NT*H] matmuls; the per-sender-j pairwise pre-relu tensor is built on
    the TensorEngine by accumulating Id@A + S_j@B into PSUM (S_j = constant
    block row-selection matrix), avoiding all elementwise broadcast adds.
  * w = sigmoid(..)*softmax(..) >= 0, so relu(w*x) = w*relu(x): the weighted
    aggregation over senders j folds into one scale+relu op per (j, type)
    (ACT/DVE alternating) followed by identity-matmul PSUM accumulation.
  * All activation transposes (for matmul lhsT operands) are PE transposes.
  * Biases are folded into matmul groups as rank-1 ones x bias_row updates.

Precision: matmul operands are bf16 (PE full rate); PSUM accumulation, the
slots state (fp32 master + bf16 mirror), the residual update add, and the
sigmoid/softmax/edge-weight path are fp32.
"""

import os

import ml_dtypes
import numpy as np

import concourse.bacc as bacc
import concourse.tile as tile
from concourse import mybir
from concourse import bass_utils

B, T, K, D, H, NT = 128, 9, 8, 8, 256, 8
NCORES = 8
BL = B // NCORES          # local batch per core
R = BL * K                # 128 rows per core
TH = NT * H               # 2048
F32 = mybir.dt.float32
BF16 = mybir.dt.bfloat16
AF = mybir.ActivationFunctionType
ALU = mybir.AluOpType
AX = mybir.AxisListType
BF = ml_dtypes.bfloat16

_build_cache: dict[int, object] = {}


def _build(steps: int, wb: bool):
    nc = bacc.Bacc("TRN2")

    def dram_in(name, shape, dt=BF16):
        return nc.dram_tensor(name, list(shape), dt, kind="ExternalInput").ap()

    def dram_out(name, shape):
        return nc.dram_tensor(name, list(shape), F32, kind="ExternalOutput").ap()

    # ---- DRAM tensors (bf16 unless noted) ---------------------------------
    d = {}
    d["x0T"] = dram_in("x0T", [D, R])
    d["idn"] = dram_in("idn", [128, 128])
    d["sel"] = dram_in("sel", [K, 128, 128])
    d["ones1"] = dram_in("ones1", [1, 128])
    d["eyemask"] = dram_in("eyemask", [128, K], F32)
    d["enc_w1"] = dram_in("enc_w1", [D, H])
    d["enc_b1r"] = dram_in("enc_b1r", [1, H])
    d["enc_w2"] = dram_in("enc_w2", [2, 128, H])
    d["enc_b2r"] = dram_in("enc_b2r", [1, H])
    d["gw1top"] = dram_in("gw1top", [2, 128, H])
    d["gw1bot"] = dram_in("gw1bot", [2, 128, H])
    d["gb1r"] = dram_in("gb1r", [1, H])
    d["gwet"] = dram_in("gwet", [2, 128, 9])
    d["gbet72"] = dram_in("gbet72", [1, 72])
    d["wm1top"] = dram_in("wm1top", [2, 128, TH])
    d["wm1bot"] = dram_in("wm1bot", [2, 128, TH])
    d["bm1r"] = dram_in("bm1r", [1, TH])
    d["wm2f"] = dram_in("wm2f", [16, 128, H])
    d["bm2"] = dram_in("bm2", [NT, H])
    d["wu1top"] = dram_in("wu1top", [2, 128, H])
    d["wu1bot"] = dram_in("wu1bot", [2, 128, H])
    d["bu1r"] = dram_in("bu1r", [1, H])
    d["wu2"] = dram_in("wu2", [2, 128, H])
    d["bu2r"] = dram_in("bu2r", [1, H])
    d["dw1"] = dram_in("dw1", [2, 128, H])
    d["db1r"] = dram_in("db1r", [1, H])
    d["dw2"] = dram_in("dw2", [2, 128, D])
    d["db2r"] = dram_in("db2r", [1, D])

    preds_o = dram_out("preds_o", [steps, R, D])
    eps_o = dram_out("eps_o", [steps, R, K])
    ets_o = dram_out("ets_o", [steps, R, K, NT])

    with tile.TileContext(nc) as tc:
        with (
            tc.tile_pool(name="wp", bufs=1) as wp,
            tc.tile_pool(name="db", bufs=3) as db,
            tc.tile_pool(name="tri", bufs=8) as tri,
            tc.tile_pool(name="st", bufs=18) as st,
            tc.tile_pool(name="psA", bufs=6, space="PSUM") as psA,
            tc.tile_pool(name="psS", bufs=2, space="PSUM") as psS,
        ):
            # ---- load constants + weights into SBUF -----------------------
            # DRAM chunked weights are [n_chunk, 128, N]; SBUF wants the
            # 128-row dim on partitions: tile [128, n_chunk, N].
            chunked = {"enc_w2", "gw1top", "gw1bot", "gwet", "wm1top",
                       "wm1bot", "wm2f", "wu1top", "wu1bot", "wu2",
                       "dw1", "dw2", "sel"}
            w = {}
            for name, ap in d.items():
                dt = F32 if name == "eyemask" else BF16
                if name in chunked:
                    n_c, p, fn = ap.shape
                    tl = wp.tile([p, n_c, fn], dt, tag=name, name=f"w_{name}")
                    nc.sync.dma_start(tl[...],
                                      ap[...].rearrange("c p n -> p c n"))
                else:
                    tl = wp.tile(list(ap.shape), dt, tag=name,
                                 name=f"w_{name}")
                    nc.sync.dma_start(tl[...], ap[...])
                w[name] = tl

            idn = w["idn"]

            # engine round-robin for PSUM->SBUF copies
            _cp = [0]

            def copy(dst, src, eng=None):
                if eng is None:
                    _cp[0] ^= 1
                    eng = "a" if _cp[0] else "v"
                if eng == "a":
                    nc.scalar.copy(dst, src)
                else:
                    nc.vector.tensor_copy(dst, src)

            def mm(out, lhsT, rhs, start, stop):
                nc.tensor.matmul(out, lhsT, rhs, start=start, stop=stop)

            def tr(out, in_):
                # bf16 PE transpose; out PSUM tile must be bf16
                nc.tensor.transpose(out, in_, idn[...])

            def bias_mm(out_ap, bias_tile, cols, stop=True):
                # out_ap += ones^T @ bias_row  (broadcast bias over partitions)
                mm(out_ap, w["ones1"][...], bias_tile[:, cols], False, stop)

            def transpose_to(dst2, src, n_chunks, name):
                # src [128, n*128] bf16 sbuf -> dst2 [128, n, 128]
                for c in range(n_chunks):
                    tp = psA.tile([128, 128], BF16, tag="psA",
                                  name=f"tp_{name}_{c}")
                    tr(tp[...], src[:, c * 128:(c + 1) * 128])
                    copy(dst2[:, c, :], tp[...])

            # ---- initial encode: slots = enc(x) ---------------------------
            def encode(inT, name):
                # inT: [D, 128] bf16 sbuf (pre-transposed input rows)
                eh_ps = psA.tile([128, H], F32, tag="psA", name=f"ehp_{name}")
                mm(eh_ps[...], inT, w["enc_w1"][...], True, not wb)
                if wb:
                    bias_mm(eh_ps[...], w["enc_b1r"], slice(0, H))
                eh = db.tile([128, H], BF16, tag="eh", name=f"eh_{name}")
                nc.scalar.activation(eh[...], eh_ps[...], AF.Relu)
                ehT = db.tile([128, 2, 128], BF16, tag="ehT",
                              name=f"ehT_{name}")
                transpose_to(ehT, eh[...], 2, f"ehT_{name}")
                sl_ps = psA.tile([128, H], F32, tag="psA", name=f"slp_{name}")
                for c in range(2):
                    mm(sl_ps[...], ehT[:, c, :], w["enc_w2"][:, c, :],
                       c == 0, c == 1 and not wb)
                if wb:
                    bias_mm(sl_ps[...], w["enc_b2r"], slice(0, H))
                slots = db.tile([128, H], F32, tag="slots",
                                name=f"slots_{name}")
                slots_bf = db.tile([128, H], BF16, tag="slots_bf",
                                   name=f"slotsbf_{name}")
                copy(slots[...], sl_ps[...])
                copy(slots_bf[...], sl_ps[...])
                return slots, slots_bf

            # one-time broadcast of discover bias row to all partitions
            gbet_bc = None
            if wb:
                gbet_ps = psA.tile([128, 72], F32, tag="psA", name="gbet_ps")
                mm(gbet_ps[...], w["ones1"][...], w["gbet72"][:, 0:72],
                   True, True)
                gbet_bc = wp.tile([128, 72], F32, tag="gbet_bc",
                                  name="gbet_bc")
                copy(gbet_bc[...], gbet_ps[...])

            slots, slots_bf = encode(w["x0T"][...], "init")

            for s in range(steps):
                # ---------------- discover -------------------------------
                slotsT = db.tile([128, 2, 128], BF16, tag="slotsT",
                                 name=f"slT_d{s}")
                transpose_to(slotsT, slots_bf[...], 2, f"d{s}")

                def lin2(rhs_t, bias_t, tag, name):
                    ps = psA.tile([128, H], F32, tag="psA", name=f"ps_{name}")
                    for c in range(2):
                        mm(ps[...], slotsT[:, c, :], rhs_t[:, c, :],
                           c == 0, c == 1 and (bias_t is None or not wb))
                    if bias_t is not None and wb:
                        bias_mm(ps[...], bias_t, slice(0, H))
                    sb = db.tile([128, H], BF16, tag=tag, name=f"sb_{name}")
                    copy(sb[...], ps[...])
                    return sb

                P = lin2(w["gw1top"], w["gb1r"], "Pd", f"P{s}")
                Q = lin2(w["gw1bot"], None, "Qd", f"Q{s}")

                h_all = db.tile([128, TH], BF16, tag="h_all",
                                name=f"hall{s}")
                for q in range(4):
                    h_ps = psS.tile([128, 512], F32, tag="ps_s",
                                    name=f"hap{s}_{q}")
                    for jj in range(2):
                        j = q * 2 + jj
                        hp = h_ps[:, jj * 256:jj * 256 + 256]
                        mm(hp, idn[...], P[...], True, False)
                        mm(hp, w["sel"][:, j, :], Q[...], False, True)
                    nc.scalar.activation(h_all[:, q * 512:(q + 1) * 512],
                                         h_ps[...], AF.Relu)
                logits_sb = db.tile([128, 72], F32, tag="logits_sb",
                                    name=f"lsb{s}")
                for j in range(K):
                    hT = db.tile([128, 2, 128], BF16, tag="hTd",
                                 name=f"hT{s}_{j}")
                    transpose_to(hT,
                                 h_all[:, j * 256:(j + 1) * 256], 2,
                                 f"h{s}_{j}")
                    l_ps = psA.tile([128, 9], F32, tag="psA",
                                    name=f"lps{s}_{j}")
                    for c in range(2):
                        mm(l_ps[...], hT[:, c, :],
                           w["gwet"][:, c, :], c == 0, c == 1)
                    copy(logits_sb[:, j * 9:(j + 1) * 9], l_ps[...])
                if wb:
                    logits = db.tile([128, 72], F32, tag="logits",
                                     name=f"lg{s}")
                    nc.vector.tensor_add(logits[...], logits_sb[...],
                                         gbet_bc[...])
                else:
                    logits = logits_sb

                lj = logits[...].rearrange("p (j c) -> p j c", c=9)
                # sigmoid(x) = 1/(1+exp(-x)) -- Exp keeps ACT on one table set
                en = db.tile([128, K], F32, tag="en", name=f"en{s}")
                nc.scalar.activation(en[...], lj[:, :, 0], AF.Exp, scale=-1.0)
                ep1 = db.tile([128, K], F32, tag="ep1", name=f"ep1{s}")
                nc.vector.tensor_scalar(ep1[...], en[...], 1.0, None,
                                        op0=ALU.add)
                ep_raw = db.tile([128, K], F32, tag="ep_raw", name=f"epr{s}")
                nc.vector.reciprocal(ep_raw[...], ep1[...])
                ep = db.tile([128, K], F32, tag="ep", name=f"ep{s}")
                nc.vector.tensor_mul(ep[...], ep_raw[...], w["eyemask"][...])
                ex = db.tile([128, K, NT], F32, tag="ex", name=f"ex{s}")
                nc.scalar.activation(ex[...], lj[:, :, 1:9], AF.Exp)
                sumt = db.tile([128, K], F32, tag="sumt", name=f"sumt{s}")
                nc.vector.reduce_sum(sumt[...], ex[...], axis=AX.X)
                rsum = db.tile([128, K], F32, tag="rsum", name=f"rsum{s}")
                nc.vector.reciprocal(rsum[...], sumt[...])
                et = db.tile([128, K, NT], F32, tag="et", name=f"et{s}")
                wv = db.tile([128, K, NT], F32, tag="wv", name=f"wv{s}")
                rs_b = rsum[...].unsqueeze(2).broadcast_to([128, K, NT])
                ep_b = ep[...].unsqueeze(2).broadcast_to([128, K, NT])
                nc.vector.tensor_mul(et[...], ex[...], rs_b)
                nc.vector.tensor_mul(wv[...], et[...], ep_b)
                nc.sync.dma_start(eps_o[s], ep[...])
                nc.sync.dma_start(ets_o[s], et[...])

                # ---------------- dynamics (2 message-passing iters) ------
                for it in range(2):
                    if it == 1:
                        slotsT = db.tile([128, 2, 128], BF16, tag="slotsT",
                                         name=f"slT_{s}_{it}")
                        transpose_to(slotsT, slots_bf[...], 2, f"y{s}_{it}")

                    def big_lin(rhs_t, bias_t, tag, name):
                        sb = db.tile([128, TH], BF16, tag=tag,
                                     name=f"bl_{name}")
                        for q in range(4):
                            ps = psA.tile([128, 512], F32, tag="psA",
                                          name=f"blp_{name}_{q}")
                            cs = slice(q * 512, (q + 1) * 512)
                            for c in range(2):
                                mm(ps[...], slotsT[:, c, :],
                                   rhs_t[:, c, cs], c == 0,
                                   c == 1 and (bias_t is None or not wb))
                            if bias_t is not None and wb:
                                bias_mm(ps[...], bias_t, cs)
                            copy(sb[:, cs], ps[...])
                        return sb

                    A = big_lin(w["wm1top"], w["bm1r"], "Adyn", f"A{s}_{it}")
                    Bt = big_lin(w["wm1bot"], None, "Bdyn", f"B{s}_{it}")

                    if wb:
                        agg_ps = psA.tile([128, H], F32, tag="psA",
                                          name=f"aggp{s}_{it}")
                    sTs = []
                    s_sbs = []
                    for half in range(2):
                        s_ps = [psS.tile([128, 512], F32, tag="ps_s",
                                         name=f"s_ps_{s}_{it}_{half}_{qq}")
                                for qq in range(2)]
                        for j in range(K):
                            for qq in range(2):
                                q = half * 2 + qq
                                cs = slice(q * 512, (q + 1) * 512)
                                pair = psA.tile([128, 512], F32, tag="psA",
                                                name=f"pair{s}_{it}_{j}_{q}")
                                mm(pair[...], idn[...], A[:, cs], True, False)
                                mm(pair[...], w["sel"][:, j, :], Bt[:, cs],
                                   False, True)
                                hw = tri.tile([128, 512], BF16, tag="hw",
                                              name=f"hw{s}_{it}_{j}_{q}")
                                on_act = (j + q) % 2 == 0
                                for tt in range(2):
                                    t = q * 2 + tt
                                    colsl = slice(tt * 256, (tt + 1) * 256)
                                    wcol = wv[:, j, t:t + 1]
                                    if on_act:
                                        nc.scalar.activation(
                                            hw[:, colsl], pair[:, colsl],
                                            AF.Relu, scale=wcol)
                                    else:
                                        nc.vector.tensor_scalar(
                                            hw[:, colsl], pair[:, colsl],
                                            wcol, 0.0, op0=ALU.mult,
                                            op1=ALU.max)
                                mm(s_ps[qq][...], idn[...], hw[...],
                                   j == 0, j == K - 1)

                        s_sb = db.tile([128, 1024], BF16, tag="s_sb",
                                       name=f"ssb{s}_{it}_{half}")
                        for qq in range(2):
                            copy(s_sb[:, qq * 512:(qq + 1) * 512],
                                 s_ps[qq][...])
                        s_sbs.append(s_sb)
                    for c in range(16):
                        s_sb = s_sbs[c // 8]
                        cc = c % 8
                        sT_ps = psA.tile([128, 128], BF16, tag="psA",
                                         name=f"sTp{s}_{it}_{c}")
                        tr(sT_ps[...], s_sb[:, cc * 128:(cc + 1) * 128])
                        sT = st.tile([128, 128], BF16, tag="sT",
                                     name=f"sT{s}_{it}_{c}")
                        copy(sT[...], sT_ps[...], "v")
                        sTs.append(sT)

                    # agg = s @ wm2f (+ (sum_j w) @ bm2 when biases exist)
                    if wb:
                        wsum = db.tile([128, NT], F32, tag="wsum",
                                       name=f"ws{s}_{it}")
                        nc.vector.reduce_sum(
                            wsum[...], wv[...].rearrange("p j t -> p t j"),
                            axis=AX.X)
                        wsum_bf = db.tile([128, NT], BF16, tag="wsum_bf",
                                          name=f"wsb{s}_{it}")
                        copy(wsum_bf[...], wsum[...])
                        wsT_ps = psA.tile([8, 128], BF16, tag="psA",
                                          name=f"wsTp{s}_{it}")
                        tr(wsT_ps[...], wsum_bf[...])
                        wsT = db.tile([8, 128], BF16, tag="wsT",
                                      name=f"wsT{s}_{it}")
                        copy(wsT[...], wsT_ps[...])

                    aggT = db.tile([128, 2, 128], BF16, tag="aggT",
                                   name=f"aggT{s}_{it}")
                    if wb:
                        for c in range(16):
                            mm(agg_ps[...], sTs[c][...], w["wm2f"][:, c, :],
                               c == 0, False)
                        mm(agg_ps[...], wsT[...], w["bm2"][...], False, True)
                        agg = db.tile([128, H], BF16, tag="agg",
                                      name=f"agg{s}_{it}")
                        copy(agg[...], agg_ps[...])
                        transpose_to(aggT, agg[...], 2, f"ag{s}_{it}")
                    else:
                        for gh in range(2):
                            aggT_ps = psA.tile([128, 128], F32, tag="psA",
                                               name=f"aggTp{s}_{it}_{gh}")
                            for c in range(16):
                                mm(aggT_ps[...],
                                   w["wm2f"][:, c, gh * 128:(gh + 1) * 128],
                                   sTs[c][...], c == 0, c == 15)
                            copy(aggT[:, gh, :], aggT_ps[...])

                    uh_ps = psA.tile([128, H], F32, tag="psA",
                                     name=f"uhp{s}_{it}")
                    for c in range(2):
                        mm(uh_ps[...], slotsT[:, c, :], w["wu1top"][:, c, :],
                           c == 0, False)
                    for c in range(2):
                        mm(uh_ps[...], aggT[:, c, :], w["wu1bot"][:, c, :],
                           False, c == 1 and not wb)
                    if wb:
                        bias_mm(uh_ps[...], w["bu1r"], slice(0, H))
                    uh = db.tile([128, H], BF16, tag="uh", name=f"uh{s}_{it}")
                    nc.scalar.activation(uh[...], uh_ps[...], AF.Relu)
                    uhT = db.tile([128, 2, 128], BF16, tag="uhT",
                                  name=f"uhT{s}_{it}")
                    transpose_to(uhT, uh[...], 2, f"uh{s}_{it}")

                    ns_ps = psA.tile([128, H], F32, tag="psA",
                                     name=f"nsp{s}_{it}")
                    for c in range(2):
                        mm(ns_ps[...], uhT[:, c, :], w["wu2"][:, c, :],
                           c == 0, c == 1 and not wb)
                    if wb:
                        bias_mm(ns_ps[...], w["bu2r"], slice(0, H))
                    # fp32 residual: slots_new = slots + mlp_out.
                    # bf16 mirror computed in parallel from the same psum.
                    new_slots = db.tile([128, H], F32, tag="slots",
                                        name=f"slots{s}_{it}")
                    slots_bf = db.tile([128, H], BF16, tag="slots_bf",
                                       name=f"slotsbf{s}_{it}")
                    nc.vector.scalar_tensor_tensor(
                        slots_bf[...], ns_ps[...], 0.0, slots[...],
                        op0=ALU.add, op1=ALU.add)
                    nc.vector.tensor_add(new_slots[...], slots[...],
                                         ns_ps[...])
                    slots = new_slots

                # ---------------- decode + re-encode ----------------------
                slotsT = db.tile([128, 2, 128], BF16, tag="slotsT",
                                 name=f"slTd{s}")
                transpose_to(slotsT, slots_bf[...], 2, f"dec{s}")
                dh_ps = psA.tile([128, H], F32, tag="psA", name=f"dhp{s}")
                for c in range(2):
                    mm(dh_ps[...], slotsT[:, c, :], w["dw1"][:, c, :],
                       c == 0, c == 1 and not wb)
                if wb:
                    bias_mm(dh_ps[...], w["db1r"], slice(0, H))
                dh = db.tile([128, H], BF16, tag="dh", name=f"dh{s}")
                nc.scalar.activation(dh[...], dh_ps[...], AF.Relu)
                dhT = db.tile([128, 2, 128], BF16, tag="dhT", name=f"dhT{s}")
                transpose_to(dhT, dh[...], 2, f"dh{s}")
                pr_ps = psA.tile([128, D], F32, tag="psA", name=f"prp{s}")
                for c in range(2):
                    mm(pr_ps[...], dhT[:, c, :], w["dw2"][:, c, :],
                       c == 0, c == 1 and not wb)
                if wb:
                    bias_mm(pr_ps[...], w["db2r"], slice(0, D))
                pred = db.tile([128, D], F32, tag="pred", name=f"pred{s}")
                copy(pred[...], pr_ps[...])
                nc.sync.dma_start(preds_o[s], pred[...])

                if s < steps - 1:
                    pred_bf = db.tile([128, D], BF16, tag="pred_bf",
                                      name=f"predbf{s}")
                    copy(pred_bf[...], pr_ps[...])
                    pT_ps = psA.tile([D, 128], BF16, tag="psA",
                                     name=f"pTp{s}")
                    tr(pT_ps[...], pred_bf[...])
                    predT = db.tile([D, 128], BF16, tag="predT",
                                    name=f"predT{s}")
                    copy(predT[...], pT_ps[...])
                    slots, slots_bf = encode(predT[...], f"re{s}")

    nc.compile()
    return nc


def _prep_shared(i):
    """Host-side preprocessing of weights into device layouts (numpy)."""
    f = lambda x: np.ascontiguousarray(np.asarray(x, dtype=np.float32))
    bf = lambda x: np.ascontiguousarray(np.asarray(x)).astype(BF)
    sh = {}
    sh["idn"] = np.eye(128, dtype=BF)
    sel = np.zeros((K, 128, 128), dtype=np.float32)
    for j in range(K):
        for b in range(BL):
            sel[j, b * K + j, b * K:(b + 1) * K] = 1.0
    sh["sel"] = sel.astype(BF)
    sh["ones1"] = np.ones((1, 128), dtype=BF)
    eye = np.ones((128, K), dtype=np.float32)
    for b in range(BL):
        for ii in range(K):
            eye[b * K + ii, ii] = 0.0
    sh["eyemask"] = eye

    def chunks2(m):  # [256, N] -> [2, 128, N]
        return bf(f(m).reshape(2, 128, -1))

    sh["enc_w1"] = bf(i["enc_w1"])
    sh["enc_b1r"] = bf(f(i["enc_b1"])[None, :])
    sh["enc_w2"] = chunks2(i["enc_w2"])
    sh["enc_b2r"] = bf(f(i["enc_b2"])[None, :])
    gd_w1 = f(i["gd_w1"])                      # [512, 256]
    sh["gw1top"] = chunks2(gd_w1[:H])
    sh["gw1bot"] = chunks2(gd_w1[H:])
    sh["gb1r"] = bf(f(i["gd_b1"])[None, :])
    gwet = np.concatenate([f(i["gd_we"]), f(i["gd_wt"])], axis=1)  # [256, 9]
    sh["gwet"] = chunks2(gwet)
    gbet = np.concatenate([f(i["gd_be"]), f(i["gd_bt"])])          # [9]
    sh["gbet72"] = bf(np.tile(gbet, K)[None, :])
    wm1 = f(i["dyn_wm1"])                      # [NT, 2H, H]
    wm1top = wm1[:, :H, :].transpose(1, 0, 2).reshape(H, TH)
    wm1bot = wm1[:, H:, :].transpose(1, 0, 2).reshape(H, TH)
    sh["wm1top"] = bf(wm1top.reshape(2, 128, TH))
    sh["wm1bot"] = bf(wm1bot.reshape(2, 128, TH))
    sh["bm1r"] = bf(f(i["dyn_bm1"]).reshape(1, TH))
    sh["wm2f"] = bf(f(i["dyn_wm2"]).reshape(TH, H).reshape(16, 128, H))
    sh["bm2"] = bf(i["dyn_bm2"])
    wu1 = f(i["dyn_wu1"])
    sh["wu1top"] = chunks2(wu1[:H])
    sh["wu1bot"] = chunks2(wu1[H:])
    sh["bu1r"] = bf(f(i["dyn_bu1"])[None, :])
    sh["wu2"] = chunks2(i["dyn_wu2"])
    sh["bu2r"] = bf(f(i["dyn_bu2"])[None, :])
    sh["dw1"] = chunks2(i["dec_w1"])
    sh["db1r"] = bf(f(i["dec_b1"])[None, :])
    sh["dw2"] = chunks2(i["dec_w2"])
    sh["db2r"] = bf(f(i["dec_b2"])[None, :])
    return sh


LAST_RESULTS = None


def kernel(**inputs):
    global LAST_RESULTS
    gt = np.ascontiguousarray(np.asarray(inputs["gt_states"], dtype=np.float32))
    steps = min(T - 1, int(inputs["rollout_steps"]))
    if steps <= 0:
        z = np.zeros
        return (z((B, 0, K, D), np.float32), z((B, 0, K, D), np.float32),
                z((B, 0, K, K), np.float32), z((B, 0, K, K, NT), np.float32))

    sh = _prep_shared(inputs)
    in_maps = []
    for c in range(NCORES):
        x0 = gt[c * BL:(c + 1) * BL, 0].reshape(R, D)
        m = dict(sh)
        m["x0T"] = np.ascontiguousarray(x0.T).astype(BF)
        in_maps.append(m)

    wb = any(bool(np.any(np.asarray(inputs[k]))) for k in
             ("enc_b1", "enc_b2", "gd_b1", "gd_be", "gd_bt", "dyn_bm1",
              "dyn_bm2", "dyn_bu1", "dyn_bu2", "dec_b1", "dec_b2"))
    nc = _build_cache.get((steps, wb))
    if nc is None:
        nc = _build(steps, wb)
        _build_cache[(steps, wb)] = nc

    trace = bool(int(os.environ.get("KERNEL_TRACE", "0")))
    res = bass_utils.run_bass_kernel_spmd(
        nc, in_maps, core_ids=list(range(NCORES)), trace=trace)
    LAST_RESULTS = res

    preds = np.stack([r["preds_o"] for r in res.results])  # [C, s, R, D]
    eps = np.stack([r["eps_o"] for r in res.results])      # [C, s, R, K]
    ets = np.stack([r["ets_o"] for r in res.results])      # [C, s, R, K, NT]

    def to_full(x, trail):
        # [C, s, R=BL*K, ...] -> [B, s, K, ...]
        xs = x.reshape((NCORES, steps, BL, K) + trail)
        return np.ascontiguousarray(
            xs.transpose((0, 2, 1, 3) + tuple(range(4, 4 + len(trail))))
            .reshape((B, steps, K) + trail))

    pred_states = to_full(preds, (D,))
    eps_f = to_full(eps, (K,))
    ets_f = to_full(ets, (K, NT))
    target_states = np.ascontiguousarray(gt[:, 1:steps + 1])
    return (pred_states, target_states, eps_f, ets_f)
